# revision 21
# baseline (speedup 1.0000x reference)
"""Trainium2 Bass kernel for nn_AttentionModel (B=4, S=4096, E=2048) on 8 cores.

Sharding: data-parallel over batch B (4) x tensor-parallel over the E output
dim of the Q projection (2). Core c handles batch b=c//2 and scores rows
m in [h*1024, (h+1)*1024) with h=c%2. No collectives.

Two reassociations cut per-core FLOPs from 120G (qkv+scores+out) to 64.5G:

  scores = (Wq_h/sqrt(E)) @ G @ Wk^T + rank-2 bias term,
  with G = x^T x the symmetric Gram matrix: only 160/256 of its 128x128
  tiles are computed on the PE; the 96 mirror tiles are PE transposes.
  This replaces the q and k projections and the [m,S]x[S,f] score GEMM.

  out = attn @ v = (attn @ Wv) @ x^T + (attn.bv): the v projection is never
  materialized. P^T = Wv-tiles @ attn^T costs 8.6G vs v's 34.4G.

Softmax keeps exp() unnormalized through the P GEMM; 1/sum and the attn.bv
bias fold into the final eviction as per-partition scale/bias.

Bias correction (host-precomputed, r = sum_s x[s,:]):
  scores += btq (x) kr + (qr + S*btq) (x) bk,  btq = bq_h/sqrt(E),
  qr = (Wq_h/sqrt(E)) r, kr = Wk r  -- a single K=2 matmul accumulation.

Schedule keeps the PE stream dense (HAM throttles on >3.4us idle): mirror
transposes run as region-end passes over staged tiles, phase 2a is split in
half around the G (d)/(e) sweeps so the second x-half residency load hides
behind 2a compute, and region (c) reuses one PSUM pool across both column
halves so the boundary pipelines.
"""

import sys

sys.path.insert(0, "/opt/trn_rl_repo")

from contextlib import ExitStack

import numpy as np

import concourse.bass as bass
import concourse.mybir as mybir
import concourse.tile as tile
from concourse import bacc
from concourse.bass_utils import run_bass_kernel_spmd
from concourse.masks import make_identity

f32 = mybir.dt.float32
f32r = mybir.dt.float32r

B, S, E = 4, 4096, 2048
EH = E // 2          # per-core scores rows (embed half)
EB = E // 128        # 16 e-blocks
MB = EH // 128       # 8 m-blocks
ST = S // 128        # 32 s-chunks
N_CORES = 8

Act = mybir.ActivationFunctionType
Alu = mybir.AluOpType
Ax = mybir.AxisListType


def build_kernel():
    nc = bacc.Bacc("TRN2", debug=False, target_bir_lowering=False)

    x_se = nc.dram_tensor("x_se", [S, E], f32r, kind="ExternalInput")   # x[b]
    xt = nc.dram_tensor("xt", [E, S], f32r, kind="ExternalInput")       # x[b]^T
    wqt = nc.dram_tensor("wqt", [E, EH], f32r, kind="ExternalInput")    # (Wq_h/sqrtE)^T
    wkt = nc.dram_tensor("wkt", [E, E], f32r, kind="ExternalInput")     # Wk^T
    wv_t = nc.dram_tensor("wv_t", [EB, 128, EB, 128], f32r, kind="ExternalInput")
    u_in = nc.dram_tensor("u_in", [2, EH], f32r, kind="ExternalInput")  # [btq; qr+S*btq]
    w_in = nc.dram_tensor("w_in", [2, E], f32r, kind="ExternalInput")   # [kr; bk]
    bv_in = nc.dram_tensor("bv_in", [128, EB, 256], f32r, kind="ExternalInput")
    outt = nc.dram_tensor("outt", [EH, S], f32, kind="ExternalOutput")

    with tile.TileContext(nc) as tc, ExitStack() as ctx:
        dram = ctx.enter_context(tc.tile_pool(name="dram", bufs=1, space="DRAM"))
        g_d = dram.tile([EB, 128, E], f32r)   # row-band: g_d[i] = G[i-blk, :]
        at_d = dram.tile([EB, 128, EH], f32r)  # row-band: at_d[fb] = A^T[fb-blk]
        attn_d = dram.tile([MB, 128, E], f32r)        # unnormalized exp rows

        const = ctx.enter_context(tc.tile_pool(name="const", bufs=1))
        ident_f = const.tile([128, 128], f32)
        make_identity(nc, ident_f[:, :])
        ident = const.tile([128, 128], f32r)
        nc.scalar.copy(ident[:, :], ident_f[:, :])

        keep = ctx.enter_context(tc.tile_pool(name="keep", bufs=1))
        rsum_sb = keep.tile([128, MB], f32)      # 1/softmax-sum per m-block col
        abv_sb = keep.tile([128, MB], f32)       # attn@bv (unnormalized)
        biasf_sb = keep.tile([128, MB], f32)     # abv*rsum

        # ================= Phase G: symmetric Gram =================
        # direct regions (tile (row i, col j) of G):
        #   (a) i 0-3,  j 0-7    (b) i 4-7, j 4-7     (c) i 0-7, j 8-15
        #   (d1) i 8-11, j 8-11  (d2) i 8-11, j 12-15 (e) i 12-15, j 12-15
        # mirror transposes: T(a:j4-7)->(4-7,0-3); T(c)->(8-15,0-7);
        #   T(d2)->(12-15,8-11).  Mirrors run as region-end passes.
        with tc.tile_pool(name="g_tstage", bufs=4) as p_tst:
            n_ev = [0]

            def evict(p_gst, ps, i, j0, jn, tag, bufs=None):
                # ps [128, jn*128] = G[i-block, j0:j0+jn) -> g_d[j][i]
                st_ = p_gst.tile([128, jn * 128], f32r, tag="gst",
                                 name=f"gst_{tag}", bufs=bufs)
                n_ev[0] += 1
                eng = nc.scalar if n_ev[0] % 2 else nc.vector
                if eng is nc.scalar:
                    nc.scalar.copy(st_[:, :], ps[:, :])
                else:
                    nc.vector.tensor_copy(st_[:, :], ps[:, :])
                nc.sync.dma_start(
                    g_d[i, :, j0 * 128:(j0 + jn) * 128], st_[:, :])
                return st_

            def mirror(tp, st_, i, j0, js, tag):
                # write T(G[i, j]) -> g_d[i][j] for j in js (js contiguous)
                pst = tp.tile([128, 128 * len(js)], f32r, tag="tps",
                              name=f"tps_{tag}")
                for t, j in enumerate(js):
                    nc.tensor.transpose(
                        pst[:, t * 128:(t + 1) * 128],
                        st_[:, (j - j0) * 128:(j - j0 + 1) * 128],
                        ident[:, :],
                    )
                ts_ = p_tst.tile([128, 128 * len(js)], f32r, tag="tst",
                                 name=f"tst_{tag}")
                nc.vector.tensor_copy(ts_[:, :], pst[:, :])
                nc.sync.dma_start(
                    g_d[js[0]:js[0] + len(js), :,
                        i * 128:(i + 1) * 128].rearrange("j p e -> p j e"),
                    ts_[:, :].rearrange("p (j e) -> p j e", e=128),
                )

            with tc.tile_pool(name="g_xres", bufs=1) as p_res:
                xh = p_res.tile([128, ST, EH], f32r)   # x[:, 0:1024]
                for sb in range(8):
                    nc.sync.dma_start(
                        xh[:, sb * 4:(sb + 1) * 4, :],
                        x_se[sb * 512:(sb + 1) * 512, 0:EH].rearrange(
                            "(s p) e -> p s e", p=128))

                # -- (a) + (b) sweeps, then T-a pass
                with tc.tile_pool(name="g_psa", bufs=2, space="PSUM") as psa, \
                     tc.tile_pool(name="g_sta", bufs=4) as p_sta, \
                     tc.tile_pool(name="g_psb", bufs=2, space="PSUM") as psb, \
                     tc.tile_pool(name="g_stb", bufs=2) as p_stb, \
                     tc.tile_pool(name="g_tpa", bufs=2, space="PSUM") as tpa:
                    sta = []
                    for i in range(4):
                        ps = psa.tile([128, 1024], f32, tag="ps")
                        for s in range(ST):
                            lhsT = xh[:, s, i * 128:(i + 1) * 128]
                            nc.tensor.matmul(ps[:, 0:512], lhsT,
                                             xh[:, s, 0:512],
                                             start=(s == 0), stop=False)
                            nc.tensor.matmul(ps[:, 512:1024], lhsT,
                                             xh[:, s, 512:1024],
                                             start=(s == 0), stop=(s == ST - 1))
                        sta.append(evict(p_sta, ps, i, 0, 8, f"a{i}"))
                    for i in range(4, 8):
                        ps = psb.tile([128, 512], f32, tag="ps")
                        for s in range(ST):
                            nc.tensor.matmul(
                                ps[:, :], xh[:, s, i * 128:(i + 1) * 128],
                                xh[:, s, 512:1024],
                                start=(s == 0), stop=(s == ST - 1))
                        evict(p_stb, ps, i, 4, 4, f"b{i}")
                    for i in range(4):
                        mirror(tpa, sta[i], i, 0, [4, 5, 6, 7], f"a{i}")

                # -- (c) rows 0-7 x cols 8-15: one PSUM pool, both halves
                stc = []
                with tc.tile_pool(name="g_stc", bufs=16) as p_stc:
                    with tc.tile_pool(name="g_psc", bufs=1,
                                      space="PSUM") as psc, \
                         tc.tile_pool(name="g_cs", bufs=3) as p_cs:
                        for ch in range(2):
                            pss = [psc.tile([128, 512], f32, tag=f"c{i}",
                                            name=f"psc{ch}_{i}")
                                   for i in range(8)]
                            for s in range(ST):
                                xrt = p_cs.tile([128, 512], f32r, tag="xr")
                                nc.scalar.dma_start(
                                    xrt[:, :],
                                    x_se[s * 128:(s + 1) * 128,
                                         EH + ch * 512:EH + (ch + 1) * 512])
                                for i in range(8):
                                    nc.tensor.matmul(
                                        pss[i][:, :],
                                        xh[:, s, i * 128:(i + 1) * 128],
                                        xrt[:, :],
                                        start=(s == 0), stop=(s == ST - 1))
                            for i in range(8):
                                stc.append(
                                    (evict(p_stc, pss[i], i, 8 + 4 * ch, 4,
                                           f"c{ch}_{i}"), i, 8 + 4 * ch))
                    with tc.tile_pool(name="g_tpc", bufs=2,
                                      space="PSUM") as tpc:
                        for st_, i, j0 in stc:
                            mirror(tpc, st_, i, j0,
                                   [j0, j0 + 1, j0 + 2, j0 + 3],
                                   f"c{j0}_{i}")

            # ---- 2a half 1 (fb 0-7) while xr_a loads ----
            with tc.tile_pool(name="g_xra", bufs=1) as p_xra:
                xra = p_xra.tile([128, ST, 512], f32r)  # x[:, 1024:1536]
                for sb in range(8):
                    nc.sync.dma_start(
                        xra[:, sb * 4:(sb + 1) * 4, :],
                        x_se[sb * 512:(sb + 1) * 512,
                             EH:EH + 512].rearrange("(s p) e -> p s e", p=128))
                phase_2a(nc, tc, g_d, at_d, wqt, range(0, EB // 2), "h1")

                # ---- (d1) rows 8-11 x cols 8-11, xr_b loads behind it ----
                with tc.tile_pool(name="g_xrb", bufs=1) as p_xrb:
                    xrb = p_xrb.tile([128, ST, 512], f32r)  # x[:, 1536:2048]
                    for sb in range(8):
                        nc.sync.dma_start(
                            xrb[:, sb * 4:(sb + 1) * 4, :],
                            x_se[sb * 512:(sb + 1) * 512,
                                 EH + 512:E].rearrange(
                                     "(s p) e -> p s e", p=128))
                    with tc.tile_pool(name="g_psd1", bufs=2,
                                      space="PSUM") as psd1, \
                         tc.tile_pool(name="g_std1", bufs=2) as p_std1:
                        for i in range(8, 12):
                            ps = psd1.tile([128, 512], f32, tag="ps")
                            for s in range(ST):
                                nc.tensor.matmul(
                                    ps[:, :],
                                    xra[:, s, (i - 8) * 128:(i - 7) * 128],
                                    xra[:, s, :],
                                    start=(s == 0), stop=(s == ST - 1))
                            evict(p_std1, ps, i, 8, 4, f"d1_{i}")
                    # (d2) rows 8-11 x cols 12-15 + (e) 12-15 x 12-15 + T-d2
                    with tc.tile_pool(name="g_psd2", bufs=2,
                                      space="PSUM") as psd2, \
                         tc.tile_pool(name="g_std2", bufs=4) as p_std2, \
                         tc.tile_pool(name="g_pse", bufs=2,
                                      space="PSUM") as pse, \
                         tc.tile_pool(name="g_ste", bufs=2) as p_ste, \
                         tc.tile_pool(name="g_tpd", bufs=2,
                                      space="PSUM") as tpd:
                        std2 = []
                        for i in range(8, 12):
                            ps = psd2.tile([128, 512], f32, tag="ps")
                            for s in range(ST):
                                nc.tensor.matmul(
                                    ps[:, :],
                                    xra[:, s, (i - 8) * 128:(i - 7) * 128],
                                    xrb[:, s, :],
                                    start=(s == 0), stop=(s == ST - 1))
                            std2.append(evict(p_std2, ps, i, 12, 4, f"d2_{i}"))
                        for i in range(12, 16):
                            ps = pse.tile([128, 512], f32, tag="ps")
                            for s in range(ST):
                                nc.tensor.matmul(
                                    ps[:, :],
                                    xrb[:, s, (i - 12) * 128:(i - 11) * 128],
                                    xrb[:, s, :],
                                    start=(s == 0), stop=(s == ST - 1))
                            evict(p_ste, ps, i, 12, 4, f"e{i}")
                        for t, i in enumerate(range(8, 12)):
                            mirror(tpd, std2[t], i, 12, [12, 13, 14, 15],
                                   f"d2_{i}")

        # ---- 2a half 2 (fb 8-15) ----
        phase_2a(nc, tc, g_d, at_d, wqt, range(EB // 2, EB), "h2")

        # ==== Phase 2b: scores = A^T-tiles @ wkt + bias; fused softmax ====
        with tc.tile_pool(name="b_wk", bufs=1) as p_wk, \
             tc.tile_pool(name="b_uw", bufs=1) as p_uw, \
             tc.tile_pool(name="b_at", bufs=2) as p_at, \
             tc.tile_pool(name="b_sm", bufs=4) as p_sm, \
             tc.tile_pool(name="b_psA", bufs=2, space="PSUM") as p_psA, \
             tc.tile_pool(name="b_psB", bufs=2, space="PSUM") as p_psB:
            wkt_sb = p_wk.tile([128, EB, E], f32r)
            for q in range(4):
                nc.sync.dma_start(
                    wkt_sb[:, q * 4:(q + 1) * 4, :],
                    wkt[q * 512:(q + 1) * 512, :].rearrange(
                        "(e p) f -> p e f", p=128))
            u_sb = p_uw.tile([2, EH], f32r)
            nc.sync.dma_start(u_sb[:, :], u_in[:, :])
            w_sb = p_uw.tile([2, E], f32r)
            nc.sync.dma_start(w_sb[:, :], w_in[:, :])
            for mb in range(MB):
                at_sb = p_at.tile([128, EB, 128], f32r, tag="at")
                nc.scalar.dma_start(
                    at_sb[:, :, :],
                    at_d[:, :, mb * 128:(mb + 1) * 128].rearrange(
                        "e p m -> p e m"))
                psA = p_psA.tile([128, 1024], f32, tag="psA")
                psB = p_psB.tile([128, 1024], f32, tag="psB")
                for e in range(EB):
                    lhsT = at_sb[:, e, :]
                    nc.tensor.matmul(psA[:, 0:512], lhsT, wkt_sb[:, e, 0:512],
                                     start=(e == 0), stop=False)
                    nc.tensor.matmul(psA[:, 512:1024], lhsT,
                                     wkt_sb[:, e, 512:1024],
                                     start=(e == 0), stop=False)
                    nc.tensor.matmul(psB[:, 0:512], lhsT,
                                     wkt_sb[:, e, 1024:1536],
                                     start=(e == 0), stop=False)
                    nc.tensor.matmul(psB[:, 512:1024], lhsT,
                                     wkt_sb[:, e, 1536:2048],
                                     start=(e == 0), stop=False)
                ub = u_sb[:, mb * 128:(mb + 1) * 128]
                nc.tensor.matmul(psA[:, 0:512], ub, w_sb[:, 0:512],
                                 start=False, stop=True)
                nc.tensor.matmul(psA[:, 512:1024], ub, w_sb[:, 512:1024],
                                 start=False, stop=True)
                nc.tensor.matmul(psB[:, 0:512], ub, w_sb[:, 1024:1536],
                                 start=False, stop=True)
                nc.tensor.matmul(psB[:, 512:1024], ub, w_sb[:, 1536:2048],
                                 start=False, stop=True)
                # fused softmax over the f axis (2048 = two psum tiles)
                negA = p_sm.tile([128, 1], f32, tag="negA")
                negB = p_sm.tile([128, 1], f32, tag="negB")
                nc.vector.tensor_reduce(out=negA[:, :], in_=psA[:, :],
                                        op=Alu.max, axis=Ax.X, negate=True)
                nc.vector.tensor_reduce(out=negB[:, :], in_=psB[:, :],
                                        op=Alu.max, axis=Ax.X, negate=True)
                negm = p_sm.tile([128, 1], f32, tag="negm")
                nc.vector.tensor_scalar(out=negm[:, :], in0=negA[:, :],
                                        scalar1=negB[:, 0:1], scalar2=None,
                                        op0=Alu.min)
                sumA = p_sm.tile([128, 1], f32, tag="sumA")
                sumB = p_sm.tile([128, 1], f32, tag="sumB")
                attnA = p_sm.tile([128, 1024], f32r, tag="attnA")
                attnB = p_sm.tile([128, 1024], f32r, tag="attnB")
                nc.scalar.activation(attnA[:, :], psA[:, :], Act.Exp,
                                     bias=negm[:, 0:1], scale=1.0,
                                     accum_out=sumA[:, 0:1])
                nc.scalar.activation(attnB[:, :], psB[:, :], Act.Exp,
                                     bias=negm[:, 0:1], scale=1.0,
                                     accum_out=sumB[:, 0:1])
                ssum = p_sm.tile([128, 1], f32, tag="ssum")
                nc.vector.tensor_scalar(out=ssum[:, :], in0=sumA[:, :],
                                        scalar1=sumB[:, 0:1], scalar2=None,
                                        op0=Alu.add)
                nc.vector.reciprocal(rsum_sb[:, mb:mb + 1], ssum[:, :])
                nc.sync.dma_start(attn_d[mb, :, 0:1024], attnA[:, :])
                nc.sync.dma_start(attn_d[mb, :, 1024:2048], attnB[:, :])

        # ===== Phase P: attn^T (PE transpose), P^T = Wv-tiles @ attn^T =====
        with tc.tile_pool(name="p_res", bufs=1) as p_pres:
            pT_sb = p_pres.tile([128, EB, EH], f32r)     # [e-part, eb, m]
            att_ctx = ExitStack()
            p_attres = att_ctx.enter_context(
                tc.tile_pool(name="p_attres", bufs=1))
            attnT_sb = p_attres.tile([128, EB, EH], f32r)  # [f-part, fb, m]
            with tc.tile_pool(name="p_ld", bufs=2) as p_ld, \
                 tc.tile_pool(name="p_tps", bufs=4, space="PSUM") as p_tps:
                for mb in range(MB):
                    ld = p_ld.tile([128, E], f32r, tag="ld")
                    nc.scalar.dma_start(ld[:, :], attn_d[mb])
                    for g in range(4):
                        pst = p_tps.tile([128, 512], f32r, tag="pst")
                        for t in range(4):
                            fkt = 4 * g + t
                            nc.tensor.transpose(
                                pst[:, t * 128:(t + 1) * 128],
                                ld[:, fkt * 128:(fkt + 1) * 128],
                                ident[:, :])
                        nc.vector.tensor_copy(
                            attnT_sb[:, 4 * g:4 * g + 4,
                                     mb * 128:(mb + 1) * 128],
                            pst[:, :].rearrange("p (c f) -> p c f", f=128))
            # attn @ bv (unnormalized)
            with tc.tile_pool(name="p_bvc", bufs=1) as p_bvc, \
                 tc.tile_pool(name="p_bv", bufs=2, space="PSUM") as p_bvp:
                bv_sb = p_bvc.tile([128, EB, 256], f32r)
                nc.sync.dma_start(bv_sb[:, :, :], bv_in[:, :, :])
                for mb in range(MB):
                    psbv = p_bvp.tile([128, 256], f32, tag="psbv")
                    for fkt in range(EB):
                        nc.tensor.matmul(
                            psbv[:, :],
                            attnT_sb[:, fkt, mb * 128:(mb + 1) * 128],
                            bv_sb[:, fkt, :],
                            start=(fkt == 0), stop=(fkt == EB - 1))
                    nc.vector.tensor_copy(abv_sb[:, mb:mb + 1], psbv[:, 0:1])
            with tc.tile_pool(name="p_wv", bufs=3) as p_wv, \
                 tc.tile_pool(name="p_ps", bufs=2, space="PSUM") as p_pps:
                for eb in range(EB):
                    wv_sb = p_wv.tile([128, EB, 128], f32r, tag="wv")
                    nc.scalar.dma_start(wv_sb[:, :, :], wv_t[eb])
                    ps = p_pps.tile([128, 1024], f32, tag="ps")
                    for fkt in range(EB):
                        lhsT = wv_sb[:, fkt, :]
                        nc.tensor.matmul(ps[:, 0:512], lhsT,
                                         attnT_sb[:, fkt, 0:512],
                                         start=(fkt == 0), stop=False)
                        nc.tensor.matmul(ps[:, 512:1024], lhsT,
                                         attnT_sb[:, fkt, 512:1024],
                                         start=(fkt == 0),
                                         stop=(fkt == EB - 1))
                    nc.scalar.copy(pT_sb[:, eb, :], ps[:, :])
            att_ctx.close()  # free attnT_sb before the out phase

            # ========= Phase out: out = P^T-tiles @ x^T =========
            for mb in range(MB):
                nc.vector.tensor_scalar(
                    out=biasf_sb[:, mb:mb + 1], in0=abv_sb[:, mb:mb + 1],
                    scalar1=rsum_sb[:, mb:mb + 1], scalar2=None, op0=Alu.mult)
            with tc.tile_pool(name="o_xt", bufs=2) as p_xt, \
                 tc.tile_pool(name="o_st", bufs=4) as p_ost, \
                 tc.tile_pool(name="o_ps", bufs=4, space="PSUM") as p_ops:
                for sc in range(8):
                    xt_sb = p_xt.tile([128, EB, 512], f32r, tag="xt")
                    nc.scalar.dma_start(
                        xt_sb[:, :, :],
                        xt[:, sc * 512:(sc + 1) * 512].rearrange(
                            "(e p) s -> p e s", p=128))
                    for mb in range(MB):
                        ps = p_ops.tile([128, 512], f32, tag="ps")
                        for e in range(EB):
                            nc.tensor.matmul(
                                ps[:, :],
                                pT_sb[:, e, mb * 128:(mb + 1) * 128],
                                xt_sb[:, e, :],
                                start=(e == 0), stop=(e == EB - 1))
                        osb = p_ost.tile([128, 512], f32, tag="osb")
                        nc.scalar.activation(
                            osb[:, :], ps[:, :], Act.Identity,
                            bias=biasf_sb[:, mb:mb + 1],
                            scale=rsum_sb[:, mb:mb + 1])
                        nc.sync.dma_start(
                            outt[mb * 128:(mb + 1) * 128,
                                 sc * 512:(sc + 1) * 512],
                            osb[:, :])

    nc.compile()
    return nc


def phase_2a(nc, tc, g_d, at_d, wqt, fb_range, suffix):
    """A^T[fb-rows, m] = sum_e G[e, fb]^T-tiles @ wqt[e, m] -> at_d[fb]."""
    with tc.tile_pool(name=f"a_wq{suffix}", bufs=1) as p_wq, \
         tc.tile_pool(name=f"a_g{suffix}", bufs=2) as p_g, \
         tc.tile_pool(name=f"a_st{suffix}", bufs=2) as p_ast, \
         tc.tile_pool(name=f"a_ps{suffix}", bufs=2, space="PSUM") as p_aps:
        wqt_sb = p_wq.tile([128, EB, EH], f32r, name=f"wqt_sb{suffix}")
        for q in range(4):
            nc.sync.dma_start(
                wqt_sb[:, q * 4:(q + 1) * 4, :],
                wqt[q * 512:(q + 1) * 512, :].rearrange(
                    "(e p) m -> p e m", p=128))
        for fb in fb_range:
            g_sb = p_g.tile([128, EB, 128], f32r, tag="g", name=f"g{suffix}")
            nc.scalar.dma_start(
                g_sb[:, :, :],
                g_d[:, :, fb * 128:(fb + 1) * 128].rearrange(
                    "e p f -> p e f"))
            ps = p_aps.tile([128, 1024], f32, tag="ps", name=f"ps{suffix}")
            for e in range(EB):
                lhsT = g_sb[:, e, :]
                nc.tensor.matmul(ps[:, 0:512], lhsT, wqt_sb[:, e, 0:512],
                                 start=(e == 0), stop=False)
                nc.tensor.matmul(ps[:, 512:1024], lhsT,
                                 wqt_sb[:, e, 512:1024],
                                 start=(e == 0), stop=(e == EB - 1))
            st_ = p_ast.tile([128, 1024], f32r, tag="ast",
                             name=f"ast{suffix}")
            nc.scalar.copy(st_[:, :], ps[:, :])
            nc.sync.dma_start(at_d[fb], st_[:, :])


_NC_CACHE = {}


def _get_nc():
    if "nc" not in _NC_CACHE:
        _NC_CACHE["nc"] = build_kernel()
    return _NC_CACHE["nc"]


def make_in_maps(x, Wq, bq, Wk, bk, Wv, bv):
    sc = np.float32(1.0 / np.sqrt(E))
    x = np.asarray(x, np.float32)
    Wq = np.asarray(Wq, np.float32)
    bq = np.asarray(bq, np.float32)
    Wk = np.asarray(Wk, np.float32)
    bk = np.asarray(bk, np.float32)
    Wv = np.asarray(Wv, np.float32)
    bv = np.asarray(bv, np.float32)

    wkt_s = np.ascontiguousarray(Wk.T)                      # [E, E]
    wv_tiled = np.ascontiguousarray(
        Wv.reshape(EB, 128, EB, 128).transpose(2, 1, 0, 3)  # [eb][fp][fb][e]
    )
    bv_pack = np.ascontiguousarray(
        np.broadcast_to(bv.reshape(EB, 128).T[:, :, None],
                        (128, EB, 256)))                    # [128, EB, 256]

    per_batch = []
    for b in range(B):
        xb = np.ascontiguousarray(x[b])                     # [S, E]
        xtb = np.ascontiguousarray(x[b].T)                  # [E, S]
        r = xb.sum(axis=0, dtype=np.float64).astype(np.float32)  # [E]
        kr = (Wk @ r).astype(np.float32)                    # [E]
        per_batch.append((xb, xtb, r, kr))

    in_maps = []
    for c in range(N_CORES):
        b, h = c // 2, c % 2
        xb, xtb, r, kr = per_batch[b]
        wq_h = Wq[h * EH:(h + 1) * EH, :] * sc              # [EH, E]
        wqt_h = np.ascontiguousarray(wq_h.T)                # [E, EH]
        btq = bq[h * EH:(h + 1) * EH] * sc
        qr = (wq_h @ r).astype(np.float32)
        u = np.ascontiguousarray(
            np.stack([btq, qr + np.float32(S) * btq]))      # [2, EH]
        w = np.ascontiguousarray(np.stack([kr, bk]))        # [2, E]
        in_maps.append({
            "x_se": xb,
            "xt": xtb,
            "wqt": wqt_h,
            "wkt": wkt_s,
            "wv_t": wv_tiled,
            "u_in": u,
            "w_in": w,
            "bv_in": bv_pack,
        })
    return in_maps


def run(in_maps, trace=False, **kwargs):
    nc = _get_nc()
    return run_bass_kernel_spmd(
        nc, in_maps, core_ids=list(range(N_CORES)), trace=trace, **kwargs
    )


def kernel(x, Wq, bq, Wk, bk, Wv, bv):
    in_maps = make_in_maps(x, Wq, bq, Wk, bk, Wv, bv)
    res = run(in_maps, trace=False)
    out = np.empty((B, E, S), dtype=np.float32)
    for c in range(N_CORES):
        b, h = c // 2, c % 2
        out[b, h * EH:(h + 1) * EH, :] = res.results[c]["outt"]
    return out
